# revision 27
# baseline (speedup 1.0000x reference)
"""TopK autoencoder (B=4096, D=1024, F=32768, K=64) on 8 Trainium2 NeuronCores.

v4: fp8 DoubleRow encoder with mode-grouped PE chains (per chunk: all 4
tiles' fp16 chains, then all 4 DR chains — 2 fp16<->DR weight-path
transitions instead of 8; each ~700 ns on HW).
  encoder pre = (t1 + t2 + t3) * 2^-14 accumulated in ONE psum bank:
    t1 = fp16(xh) @ fp16(Wh*2^14)          8 plain matmuls / chunk-tile
    t2 = e4m3(xh) @ e4m3(Wl*2^14)          4 DoubleRow matmuls
    t3 = e4m3(xl*2^6) @ e4m3(Wh*2^8)       4 DoubleRow matmuls
  descale folded into the ReLU activation's scale (2^-14).
  Selection error budget: ~0.2% of rows get one near-threshold swap
  (rel-err ~6e-3 vs the 2e-2 gate; exact-fp16 3-pass fallback kept for
  the b_enc != 0 case where KT is odd).
v2 structure: single sweep per rep covering all 4 row tiles per W chunk
(W_enc streamed once), selection/decode of the previous rep interleaved
as small closures between chunks, ping/pong spill buffers by rep parity.
"""
import sys
sys.path.insert(0, '/opt/trn_rl_repo')
import numpy as np
import concourse.bass as bass
import concourse.mybir as mybir
from concourse import bacc
from concourse.bass import ts, ds
from concourse.tile import TileContext
from concourse.masks import make_identity
from concourse.bass_utils import run_bass_kernel_spmd

f32 = mybir.dt.float32
f16 = mybir.dt.float16
f8e4 = mybir.dt.float8e4
u32 = mybir.dt.uint32
i32 = mybir.dt.int32
Alu = mybir.AluOpType
Act = mybir.ActivationFunctionType
AxX = mybir.AxisListType.X
DR = mybir.MatmulPerfMode.DoubleRow

B, D, F, K = 4096, 1024, 32768, 64
N_CORES = 8
GE = 32     # candidate group size
NP = 72     # candidate groups per row
FC = 512    # F-chunk width
SPLITS = ((0, 0), (1, 1), (2, 0))   # fp16x3 fallback: (xh,Wh), (xh*2^-11, Wl*2^11), (xl,Wh)

_CACHE = {}


def build(DX, DO, FF, BL, n_cores=N_CORES, reps=1, fp8=True):
    KT = DX // 128
    NT = BL // 128          # 4 row tiles
    NFC = FF // FC          # 64 chunks
    NG = FF // GE           # 1024 groups
    GPC = FC // GE          # 16 groups per chunk
    ND2 = max(1, DO // 512)
    DW = DO // ND2
    PB = 4                  # decode gather batch
    assert not fp8 or KT % 2 == 0

    nc = bacc.Bacc("TRN2", target_bir_lowering=False, debug=False, num_devices=n_cores)
    if fp8:
        xt16 = nc.dram_tensor("xt16", [128, KT, BL], f16, kind="ExternalInput")
        xt8 = nc.dram_tensor("xt8", [128, 2, KT // 2, 2, BL], f8e4, kind="ExternalInput")
        wencT16 = nc.dram_tensor("wencT16", [NFC, 128, KT, FC], f16, kind="ExternalInput")
        wenc8 = nc.dram_tensor("wenc8", [NFC, 128, 2, KT // 2, 2, FC], f8e4,
                               kind="ExternalInput")
    else:
        xt = nc.dram_tensor("xt", [128, 3, KT, BL], f16, kind="ExternalInput")
        wencT = nc.dram_tensor("wencT", [NFC, 128, 2, KT, FC], f16, kind="ExternalInput")
    wdecT = nc.dram_tensor("wdecT", [FF, DO], f16, kind="ExternalInput")
    out = nc.dram_tensor("out", [BL, DO], f32, kind="ExternalOutput")

    with TileContext(nc) as tc:
        with (
            tc.tile_pool(name="dramA", bufs=1, space="DRAM") as dpoolA,
            tc.tile_pool(name="dramB", bufs=1, space="DRAM") as dpoolB,
            tc.tile_pool(name="xt_sb", bufs=1) as xpool,
            tc.tile_pool(name="const", bufs=1) as kpool,
            tc.tile_pool(name="wenc", bufs=2) as wpool,
            tc.tile_pool(name="apsum", bufs=6, space="PSUM") as apsum,
            tc.tile_pool(name="abounce", bufs=6) as apool,
            tc.tile_pool(name="gbuf", bufs=1) as gpool,
            tc.tile_pool(name="cand", bufs=1) as cpool,
            tc.tile_pool(name="pack", bufs=1) as ppool,
            tc.tile_pool(name="packbig", bufs=1) as qpool,
            tc.tile_pool(name="small", bufs=4) as spool,
            tc.tile_pool(name="sel", bufs=2) as selpool,
            tc.tile_pool(name="wdecg", bufs=2) as wgpool,
            tc.tile_pool(name="dgal", bufs=1) as dgpool,
            tc.tile_pool(name="dpsum", bufs=1, space="PSUM") as dpsum,
            tc.tile_pool(name="cout", bufs=1) as opool,
        ):
            preDs = [dpoolA.tile([BL, FF], f32, name="preDA"),
                     dpoolB.tile([BL, FF], f32, name="preDB")]

            if fp8:
                xt16_sb = xpool.tile([128, KT, BL], f16)
                nc.sync.dma_start(out=xt16_sb[:], in_=xt16.ap())
                xt8_sb = xpool.tile([128, 2, KT // 2, 2, BL], f8e4)
                nc.sync.dma_start(out=xt8_sb[:], in_=xt8.ap())
            else:
                xt_sb = xpool.tile([128, 3, KT, BL], f16)
                nc.sync.dma_start(out=xt_sb[:], in_=xt.ap())
            ident = kpool.tile([128, 128], f16)
            make_identity(nc, ident[:])
            gid = kpool.tile([128, NG], i32)
            nc.gpsimd.iota(gid[:], pattern=[[1, NG]], base=0, channel_multiplier=0)
            tagi = kpool.tile([128, GE], i32)
            nc.gpsimd.iota(tagi[:], pattern=[[1, GE]], base=0, channel_multiplier=0)
            # per-tile candidate-gather base offsets: (t*128 + p) * NG
            goffb = []
            for t in range(NT):
                gb = kpool.tile([128, 1], i32, name=f"goffb{t}")
                nc.gpsimd.iota(gb[:], pattern=[[0, 1]], base=t * 128 * NG,
                               channel_multiplier=NG)
                goffb.append(gb)

            Gs = [[gpool.tile([128, NG], f32, name=f"G{g}{t}") for t in range(NT)]
                  for g in range(2)]

            def sweep(gen, inter):
                """Full-F encoder sweep for all NT tiles; W streamed once.

                inter: {fc: [callback, ...]} emitted after chunk fc's tiles.
                """
                preD = preDs[gen]
                for fc in range(NFC):
                    if fp8:
                        w16 = wpool.tile([128, KT, FC], f16, name="w16")
                        nc.sync.dma_start(out=w16[:], in_=wencT16.ap()[fc])
                        w8 = wpool.tile([128, 2, KT // 2, 2, FC], f8e4, name="w8")
                        nc.sync.dma_start(out=w8[:], in_=wenc8.ap()[fc])
                    else:
                        w = wpool.tile([128, 2, KT, FC], f16, name="w")
                        nc.sync.dma_start(out=w[:], in_=wencT.ap()[fc])
                    if fp8:
                        # mode-grouped: all tiles' fp16 chains, then all DR
                        # chains — 2 fp16<->DR weight-path transitions per
                        # chunk instead of 8. One full-width chain per bank;
                        # only its first matmul carries start=True (PSUM
                        # zeroing is bank-granular).
                        pss = [apsum.tile([128, FC], f32, name="ps")
                               for _ in range(NT)]
                        for t in range(NT):
                            # two same-bank 256-wide half-chains (~1.4x faster
                            # than one 512-wide chain). PSUM start zeroing is
                            # bank-granular, so ONLY the first matmul in the
                            # bank carries start=True — it pre-zeroes the h1
                            # region too; every other matmul accumulates.
                            for h in range(2):
                                sl = ds(h * (FC // 2), FC // 2)
                                for k in range(KT):
                                    nc.tensor.matmul(
                                        pss[t][:, sl], lhsT=xt16_sb[:, k, ts(t, 128)],
                                        rhs=w16[:, k, sl],
                                        start=(h == 0 and k == 0), stop=False)
                        for t in range(NT):
                            for term in range(2):
                                for j in range(KT // 2):
                                    nc.tensor.matmul(
                                        pss[t][:], lhsT=xt8_sb[:, term, j, :, ts(t, 128)],
                                        rhs=w8[:, term, j, :, :],
                                        perf_mode=DR, start=False,
                                        stop=(term == 1 and j == KT // 2 - 1))
                        for t in range(NT):
                            a = apool.tile([128, FC], f32, name="a")
                            nc.scalar.activation(a[:], pss[t][:], Act.Relu,
                                                 scale=2.0 ** -14)
                            nc.sync.dma_start(out=preD[ts(t, 128), ds(fc * FC, FC)],
                                              in_=a[:])
                            av = a[:, :].rearrange("p (g e) -> p g e", e=GE)
                            nc.vector.reduce_max(
                                out=Gs[gen][t][:, ds(fc * GPC, GPC)], in_=av, axis=AxX)
                    else:
                        for t in range(NT):
                            ps = apsum.tile([128, FC], f32, name="ps")
                            for gi, (xs_, ws_) in enumerate(SPLITS):
                                for k in range(KT):
                                    nc.tensor.matmul(
                                        ps[:], lhsT=xt_sb[:, xs_, k, ts(t, 128)],
                                        rhs=w[:, ws_, k, :],
                                        start=(gi == 0 and k == 0),
                                        stop=(gi == 2 and k == KT - 1),
                                    )
                            a = apool.tile([128, FC], f32, name="a")
                            nc.scalar.activation(a[:], ps[:], Act.Relu)
                            nc.sync.dma_start(out=preD[ts(t, 128), ds(fc * FC, FC)],
                                              in_=a[:])
                            av = a[:, :].rearrange("p (g e) -> p g e", e=GE)
                            nc.vector.reduce_max(
                                out=Gs[gen][t][:, ds(fc * GPC, GPC)], in_=av, axis=AxX)
                    for cb in inter.get(fc, ()):
                        cb()

            def make_bc(gen, t):
                """Selection+decode for tile t of rep parity `gen`, as a list of
                small closures (to be spread across the next sweep)."""
                preD = preDs[gen]
                G = Gs[gen][t]
                preD_g = preD[:, :].rearrange("b (g e) -> (b g) e", e=GE)
                st = {}

                def sel_a():
                    gpk = ppool.tile([128, NG], u32, name="gpk")
                    nc.vector.tensor_scalar(out=gpk[:], in0=G[:, :].bitcast(u32),
                                            scalar1=0xFFFF0000, scalar2=None,
                                            op0=Alu.bitwise_and)
                    nc.vector.tensor_tensor(out=gpk[:], in0=gpk[:],
                                            in1=gid[:, :].bitcast(u32),
                                            op=Alu.bitwise_or)
                    gtop = spool.tile([128, NP], f32, name="gtop")
                    gpkf = gpk[:, :].bitcast(f32)
                    for r in range(5):
                        mv = gtop[:, ds(r * 8, 8)]
                        nc.vector.max(out=mv, in_=gpkf)
                        if r < NP // 8 - 1:
                            nc.vector.match_replace(out=gpkf, in_to_replace=mv,
                                                    in_values=gpkf, imm_value=0.0)
                    st["gpk"], st["gtop"] = gpk, gtop

                def sel_b():
                    gpk, gtop = st["gpk"], st["gtop"]
                    gpkf = gpk[:, :].bitcast(f32)
                    for r in range(5, NP // 8):
                        mv = gtop[:, ds(r * 8, 8)]
                        nc.vector.max(out=mv, in_=gpkf)
                        if r < NP // 8 - 1:
                            nc.vector.match_replace(out=gpkf, in_to_replace=mv,
                                                    in_values=gpkf, imm_value=0.0)
                    gsel = spool.tile([128, NP], u32, name="gsel")
                    # NG-1 mask keeps garbage offsets in-bounds on timing runs
                    nc.vector.tensor_scalar(out=gsel[:], in0=gtop[:, :].bitcast(u32),
                                            scalar1=NG - 1, scalar2=None,
                                            op0=Alu.bitwise_and)
                    goff = spool.tile([128, NP], i32, name="goff")
                    gb_b = goffb[t][:, 0:1].to_broadcast([128, NP])
                    nc.vector.tensor_tensor(out=goff[:], in0=gsel[:, :].bitcast(i32),
                                            in1=gb_b, op=Alu.add)
                    st["gsel"], st["goff"] = gsel, goff

                def gather(lo, hi):
                    def _f():
                        if "cand" not in st:
                            st["cand"] = cpool.tile([128, NP, GE], f32, name="cand")
                        cand, goff = st["cand"], st["goff"]
                        for kk in range(lo, hi):
                            nc.gpsimd.indirect_dma_start(
                                out=cand[:, kk, :], out_offset=None, in_=preD_g,
                                in_offset=bass.IndirectOffsetOnAxis(
                                    ap=goff[:, kk:kk + 1], axis=0),
                            )
                    return _f

                def pack():
                    cand, gsel = st["cand"], st["gsel"]
                    gsel_b = gsel[:, :].rearrange("p (n o) -> p n o", o=1) \
                        .to_broadcast([128, NP, GE])
                    tagi_b = tagi[:, :].rearrange("p (o e) -> p o e", o=1) \
                        .to_broadcast([128, NP, GE])
                    tagm = ppool.tile([128, NP, GE], u32, name="tagm")
                    nc.vector.tensor_scalar(out=tagm[:], in0=gsel_b, scalar1=GE,
                                            scalar2=None, op0=Alu.mult)
                    nc.vector.tensor_tensor(out=tagm[:], in0=tagm[:],
                                            in1=tagi_b.bitcast(u32), op=Alu.add)
                    cand2 = cand[:, :, :].rearrange("p n e -> p (n e)")
                    cpk = qpool.tile([128, NP * GE], u32, name="cpk")
                    nc.vector.tensor_scalar(out=cpk[:], in0=cand2.bitcast(u32),
                                            scalar1=0xFFFF0000, scalar2=None,
                                            op0=Alu.bitwise_and)
                    tagm2 = tagm[:, :, :].rearrange("p n e -> p (n e)")
                    nc.vector.tensor_tensor(out=cpk[:], in0=cpk[:], in1=tagm2,
                                            op=Alu.bitwise_or)
                    vr = qpool.tile([128, NP * GE], f32, name="vr")
                    nc.vector.tensor_copy(vr[:], cand2)
                    st["cpk"], st["vr"] = cpk, vr

                def vr_rounds(lo, hi):
                    def _f():
                        vr = st["vr"]
                        for r in range(lo, hi):
                            mvf = spool.tile([128, 8], f32, name="mvf")
                            nc.vector.max(out=mvf[:], in_=vr[:])
                            if r < K // 8 - 1:
                                nc.vector.match_replace(out=vr[:], in_to_replace=mvf[:],
                                                        in_values=vr[:], imm_value=0.0)
                            st["mvf"] = mvf
                        if hi == K // 8:
                            cand2 = st["cand"][:, :, :].rearrange("p n e -> p (n e)")
                            tstar = spool.tile([128, 1], f32, name="tstar")
                            nc.vector.tensor_copy(tstar[:], st["mvf"][:, 7:8])
                            cpkf = st["cpk"][:, :].bitcast(f32)
                            nc.vector.scalar_tensor_tensor(
                                out=cpkf, in0=cand2, scalar=tstar[:], in1=cpkf,
                                op0=Alu.is_ge, op1=Alu.mult,
                            )
                    return _f

                def pk_rounds(lo, hi):
                    def _f():
                        cpkf = st["cpk"][:, :].bitcast(f32)
                        if "pk" not in st:
                            st["pk"] = spool.tile([128, K], f32, name="pk")
                        pk = st["pk"]
                        for r in range(lo, hi):
                            mv = pk[:, ds(r * 8, 8)]
                            nc.vector.max(out=mv, in_=cpkf)
                            if r < K // 8 - 1:
                                nc.vector.match_replace(out=cpkf, in_to_replace=mv,
                                                        in_values=cpkf, imm_value=0.0)
                        if hi == K // 8:
                            fsel = selpool.tile([128, K], u32, name="fsel")
                            nc.vector.tensor_scalar(out=fsel[:], in0=pk[:, :].bitcast(u32),
                                                    scalar1=0x7FFF, scalar2=None,
                                                    op0=Alu.bitwise_and)
                            wsel = selpool.tile([128, K], f32, name="wsel")
                            nc.vector.tensor_scalar(out=wsel[:, :].bitcast(u32),
                                                    in0=pk[:, :].bitcast(u32),
                                                    scalar1=0xFFFF0000, scalar2=None,
                                                    op0=Alu.bitwise_and)
                            st["fsel"], st["wsel"] = fsel, wsel
                    return _f

                def dec_gather(blk):
                    def _f():
                        wg = wgpool.tile([128, PB, DO], f16, name="wg")
                        st[f"wg{blk}"] = wg
                        fsel = st["fsel"]
                        for j in range(PB):
                            kk = blk * PB + j
                            nc.gpsimd.indirect_dma_start(
                                out=wg[:, j, :], out_offset=None, in_=wdecT[:, :],
                                in_offset=bass.IndirectOffsetOnAxis(
                                    ap=fsel[:, kk:kk + 1], axis=0),
                            )
                    return _f

                def dec_mm(blk0, blk1):
                    def _f():
                        if "dgall" not in st:
                            dgall = dgpool.tile([128, K, 128], f16, name="dgall")
                            wsel_b = st["wsel"][:, :].rearrange("p (k o) -> p k o", o=1) \
                                .to_broadcast([128, K, 128])
                            ident_b = ident[:, :].rearrange("p (o j) -> p o j", o=1) \
                                .to_broadcast([128, K, 128])
                            nc.vector.tensor_tensor(out=dgall[:], in0=wsel_b,
                                                    in1=ident_b, op=Alu.mult)
                            st["dgall"] = dgall
                            st["psD"] = [dpsum.tile([128, DW], f32, name=f"psD{h}")
                                         for h in range(ND2)]
                        dgall, psD = st["dgall"], st["psD"]
                        for blk in range(blk0, blk1):
                            wg = st[f"wg{blk}"]
                            for j in range(PB):
                                kk = blk * PB + j
                                for h in range(ND2):
                                    # same-bank 256-wide half-chains; single
                                    # start=True per bank (q==0 pre-zeroes q==1)
                                    for q in range(2):
                                        nc.tensor.matmul(
                                            psD[h][:, ds(q * (DW // 2), DW // 2)],
                                            lhsT=dgall[:, kk, :],
                                            rhs=wg[:, j, ds(h * DW + q * (DW // 2),
                                                            DW // 2)],
                                            start=(kk == 0 and q == 0),
                                            stop=(kk == K - 1 and q == 1))
                        if blk1 == K // PB:
                            co = opool.tile([128, DO], f32, name="co")
                            for h in range(ND2):
                                nc.vector.tensor_copy(co[:, ds(h * DW, DW)], psD[h][:])
                            nc.sync.dma_start(out=out.ap()[ts(t, 128), :], in_=co[:])
                    return _f

                NB = K // PB  # decode gather blocks
                cls = [
                    sel_a, sel_b,
                    gather(0, 24), gather(24, 48), gather(48, NP),
                    pack,
                    vr_rounds(0, 4), vr_rounds(4, K // 8),
                    pk_rounds(0, 4), pk_rounds(4, K // 8),
                    dec_gather(0), dec_gather(1),
                ]
                for blk in range(2, NB):
                    cls += [dec_mm(blk - 2, blk - 1), dec_gather(blk)]
                cls += [dec_mm(NB - 2, NB - 1), dec_mm(NB - 1, NB)]
                return cls

            def bc_map(gen):
                """Spread the 4 tiles' B/C closures across the 64 sweep chunks."""
                m = {}
                for t in range(NT):
                    cls = make_bc(gen, t)
                    base = 1 + t * 15
                    for i, cb in enumerate(cls):
                        fc = base + int(i * 14 / max(1, len(cls) - 1))
                        m.setdefault(min(fc, NFC - 1), []).append(cb)
                return m

            def tail_bc(gen):
                for t in range(NT):
                    for cb in make_bc(gen, t):
                        cb()

            if reps == 1:
                sweep(0, {})
                tail_bc(0)
            else:
                assert reps % 2 == 0, "reps must be even"
                sweep(0, {})
                if reps > 2:
                    with tc.For_i(0, (reps - 2) // 2, 1):
                        sweep(1, bc_map(0))
                        sweep(0, bc_map(1))
                sweep(1, bc_map(0))
                tail_bc(1)

    nc.compile()
    return nc


def get_kernel(DX, reps=1, fp8=True):
    key = (DX, reps, fp8)
    if key not in _CACHE:
        _CACHE[key] = build(DX, D, F, B // N_CORES, N_CORES, reps=reps, fp8=fp8)
    return _CACHE[key]


def prep_in_maps(x, W_enc, b_enc, W_dec, b_dec):
    import ml_dtypes
    E4 = mybir.dt.np(mybir.dt.float8e4)
    BL = B // N_CORES
    xs = (x - b_dec).astype(np.float32)
    wencT = np.ascontiguousarray(W_enc.T.astype(np.float32))   # [D, F]
    if np.any(b_enc):
        # fold b_enc in as one extra 128-row contraction tile (fp16x3 path)
        DX = D + 128
        xa = np.zeros((B, DX), np.float32)
        xa[:, :D] = xs
        xa[:, D] = 1.0
        wa = np.zeros((DX, F), np.float32)
        wa[:D] = wencT
        wa[D] = b_enc
        xs, wencT = xa, wa
        fp8 = False
    else:
        DX = D
        fp8 = True
    KT = DX // 128
    NFC = F // FC
    xst = np.ascontiguousarray(xs.T)                            # [DX, B]
    wdecT = np.ascontiguousarray(W_dec.T).astype(np.float16)    # [F, D]
    xh32 = xst.astype(np.float16).astype(np.float32)
    wh32 = wencT.astype(np.float16).astype(np.float32)
    xl32 = xst - xh32
    wl32 = wencT - wh32
    if fp8:
        # fp16 hi term, scaled 2^14 on the W side
        whs = (wh32 * 2.0 ** 14).astype(np.float16)             # [DX, F]
        wencT16 = np.ascontiguousarray(
            whs.reshape(KT, 128, NFC, FC).transpose(2, 1, 0, 3))
        xh16 = xh32.astype(np.float16)                          # [DX, B]
        xt16 = np.ascontiguousarray(xh16.reshape(KT, 128, B).transpose(1, 0, 2))
        # fp8 terms (DoubleRow k-pair layout)
        wl14 = (wl32 * 2.0 ** 14).astype(E4)
        wh8 = (wh32 * 2.0 ** 8).astype(E4)
        w8 = np.stack([wl14, wh8])                              # [2, DX, F]
        wenc8 = np.ascontiguousarray(
            w8.reshape(2, KT // 2, 2, 128, NFC, FC).transpose(4, 3, 0, 1, 2, 5))
        xh8 = xh32.astype(E4)
        xl6 = (xl32 * 2.0 ** 6).astype(E4)
        x8 = np.stack([xh8, xl6])                               # [2, DX, B]
        xt8 = np.ascontiguousarray(
            x8.reshape(2, KT // 2, 2, 128, B).transpose(3, 0, 1, 2, 4))
        in_maps = [{
            "xt16": np.ascontiguousarray(xt16[:, :, c * BL:(c + 1) * BL]),
            "xt8": np.ascontiguousarray(xt8[:, :, :, :, c * BL:(c + 1) * BL]),
            "wencT16": wencT16,
            "wenc8": wenc8,
            "wdecT": wdecT,
        } for c in range(N_CORES)]
    else:
        wls = (wl32 * 2.0 ** 11).astype(np.float16)
        wenc2 = np.stack([wh32.astype(np.float16), wls])        # [2, DX, F]
        wenc_l = np.ascontiguousarray(
            wenc2.reshape(2, KT, 128, NFC, FC).transpose(3, 2, 0, 1, 4))
        xh = xh32.astype(np.float16)
        xhs = (xh32 * 2.0 ** -11).astype(np.float16)
        xl = xl32.astype(np.float16)
        xt3 = np.stack([xh, xhs, xl])                           # [3, DX, B]
        xt_l = np.ascontiguousarray(xt3.reshape(3, KT, 128, B).transpose(2, 0, 1, 3))
        in_maps = [{
            "xt": np.ascontiguousarray(xt_l[:, :, :, c * BL:(c + 1) * BL]),
            "wencT": wenc_l,
            "wdecT": wdecT,
        } for c in range(N_CORES)]
    return in_maps, DX, fp8


def kernel(x, W_enc, b_enc, W_dec, b_dec):
    x = np.asarray(x, np.float32)
    W_enc = np.asarray(W_enc, np.float32)
    b_enc = np.asarray(b_enc, np.float32)
    W_dec = np.asarray(W_dec, np.float32)
    b_dec = np.asarray(b_dec, np.float32)
    in_maps, DX, fp8 = prep_in_maps(x, W_enc, b_enc, W_dec, b_dec)
    nc = get_kernel(DX, fp8=fp8)
    res = run_bass_kernel_spmd(nc, in_maps, list(range(N_CORES)))
    y = np.concatenate([res.results[c]["out"] for c in range(N_CORES)], axis=0)
    return (y + b_dec).astype(np.float32)
